# revision 1
# baseline (speedup 1.0000x reference)
"""Dilated attention (LongNet-style) Bass kernel for 8 Trainium2 NeuronCores.

Problem: q,k,v of shape (B=2, S=8192, H=16, D=64) fp32.
4 head-groups x (segment length s, dilation r) with s/r == 1024 for every
group, so the whole computation is 120 identical 1024x1024x64 attention
sub-problems plus a per-(batch, head, channel) sum-normalization.

Sharding: core = b*4 + j owns heads {j, 4+j, 8+j, 12+j} of batch b, i.e.
one head from each group -> 8+4+2+1 = 15 sub-problems per core (perfectly
balanced), and every (batch, head) lives on exactly one core so the
normalization is core-local.

Numerics: the final x / sum(x) normalization is badly conditioned (sums
cancel to ~1e-2 of their element magnitudes on some heads), so 16-bit
matmul inputs are not enough. q/k and V are fed as fp16 hi+lo pairs
(~21 effective mantissa bits); the exp'd scores are single fp16.

Per sub-problem on-device (fp32 PSUM accumulation):
  S^T[k,q] = khi.T(qhi+qlo) + klo.T qhi    (K=128 stacked MM + row-packed
                                            64x128 correction MM)
  E        = exp(S^T) in fp16              (softmax scale folded into q)
  O'[d,q]  = [Vhi|1].T E + [Vlo|0].T E     (row 64 = softmax denominator l)
  x        = O'[0:64] * (1/l)              (recip row broadcast via DRAM DMA)
  out      = x / (4 * sum_{segs,q} x)      per (head, channel)
"""

import os
import numpy as np
import ml_dtypes

import concourse.bass as bass
import concourse.bacc as bacc
import concourse.mybir as mybir
import concourse.tile as tile
from concourse import bass_utils

# ---------------------------------------------------------------- constants
B, S, H, D = 2, 8192, 16, 64
SEGMENT_LENGTHS = [1024, 2048, 4096, 8192]
DILATION_RATES = [1, 2, 4, 8]
NUM_GROUPS = 4
GROUP_HEADS = H // NUM_GROUPS  # 4
SEGS_PER_GROUP = [S // s for s in SEGMENT_LENGTHS]  # [8, 4, 2, 1]
NPROB = sum(SEGS_PER_GROUP)  # 15 problems per core
SL = 1024          # per-problem sequence length (s // r, same for all groups)
NCHUNK = SL // 128  # 8 key chunks
N_CORES = 8
SCALE = 1.0 / np.sqrt(D)

BF16 = mybir.dt.bfloat16
FP32 = mybir.dt.float32
FP16 = mybir.dt.float16
VW = D + 1  # 65: V plus the ones column


def _problem_list(j):
    """15 (group, head, seg) tuples for local head-slot j, head-contiguous."""
    out = []
    for g in range(NUM_GROUPS):
        head = g * GROUP_HEADS + j
        for seg in range(SEGS_PER_GROUP[g]):
            out.append((g, head, seg))
    return out


def _positions(g, seg):
    s, r = SEGMENT_LENGTHS[g], DILATION_RATES[g]
    offset = g % r
    return seg * s + offset + r * np.arange(SL)


# ---------------------------------------------------------------- device IR
def _build_tile_program(ctx, tc, out_ap, qka_ap, qkb_ap, vp_ap):
    nc = tc.nc
    EXP = mybir.ActivationFunctionType.Exp

    qka_pool = ctx.enter_context(tc.tile_pool(name="qka", bufs=3))
    qkb_pool = ctx.enter_context(tc.tile_pool(name="qkb", bufs=3))
    vp_pool = ctx.enter_context(tc.tile_pool(name="vp", bufs=3))
    exp_pool = ctx.enter_context(tc.tile_pool(name="exps", bufs=3))
    sout_pool = ctx.enter_context(tc.tile_pool(name="sout", bufs=3))
    snorm_pool = ctx.enter_context(tc.tile_pool(name="snorm", bufs=11))
    rrow_pool = ctx.enter_context(tc.tile_pool(name="rrow", bufs=3))
    sums_pool = ctx.enter_context(tc.tile_pool(name="sums", bufs=6))
    fin_pool = ctx.enter_context(tc.tile_pool(name="fin", bufs=3))
    rlb_pool = ctx.enter_context(tc.tile_pool(name="rlb", bufs=2))
    rdram_pool = ctx.enter_context(
        tc.tile_pool(name="rdram", bufs=2, space="DRAM"))
    spsum = ctx.enter_context(tc.tile_pool(name="spsum", bufs=2, space="PSUM"))
    pvpsum = ctx.enter_context(tc.tile_pool(name="pvpsum", bufs=2, space="PSUM"))

    # per-problem state; problems are head-contiguous
    probs = []
    for g in range(NUM_GROUPS):
        for seg in range(SEGS_PER_GROUP[g]):
            probs.append({
                "first": seg == 0,
                "last": seg == SEGS_PER_GROUP[g] - 1,
            })
    for p, st in enumerate(probs):
        st["p"] = p
    head_lists = []
    i = 0
    for nseg in SEGS_PER_GROUP:
        head_lists.append(probs[i:i + nseg])
        i += nseg
    for hl in head_lists:
        for st in hl:
            st["head_list"] = hl

    def emit_bcast(st):
        # broadcast the 1/l row to 64 partitions: SBUF -> DRAM -> stride-0 DMA
        r_d = rdram_pool.tile([1, SL], FP32)
        nc.gpsimd.dma_start(out=r_d, in_=st["r_row"])
        rl_b = rlb_pool.tile([D, SL], FP32)
        st["rl_b"] = rl_b
        src = bass.AP(tensor=r_d.tensor, offset=r_d.offset,
                      ap=[[0, D]] + [list(d) for d in r_d.ap[1:]])
        nc.gpsimd.dma_start(out=rl_b, in_=src)

    def emit_norm(st):
        # s_norm = s_out[0:64] * bcast(1/l); seg_sum = sum_q s_norm + prev
        prev_accum = None if st["first"] else probs[st["p"] - 1]["seg_sum"]
        s_norm = snorm_pool.tile([D, SL], FP32)
        seg_local = sums_pool.tile([D, 1], FP32, tag="seg_local")
        nc.vector.tensor_mul(s_norm, st["s_out"][0:D, :], st["rl_b"])
        nc.vector.reduce_sum(seg_local, s_norm, axis=mybir.AxisListType.X)
        if prev_accum is None:
            seg_sum = seg_local
        else:
            seg_sum = sums_pool.tile([D, 1], FP32, tag="seg_sum")
            nc.vector.tensor_add(seg_sum, seg_local, prev_accum)
        st["s_norm"] = s_norm
        st["seg_sum"] = seg_sum
        if st["last"]:
            emit_head_finals(st)

    def emit_head_finals(last_st):
        # rh = 1 / (4 * head_sum); out = s_norm * rh, DMA out
        hs4 = sums_pool.tile([D, 1], FP32)
        nc.vector.tensor_scalar_mul(hs4, last_st["seg_sum"], float(NUM_GROUPS))
        rh = sums_pool.tile([D, 1], FP32)
        nc.vector.reciprocal(out=rh, in_=hs4)
        for st in last_st["head_list"]:
            fin = fin_pool.tile([D, SL], FP32)
            nc.vector.tensor_scalar_mul(fin, st["s_norm"], rh)
            nc.gpsimd.dma_start(out=out_ap[st["p"]], in_=fin)

    prev = None  # previous problem (epilogue pipelined one problem behind)

    reps = int(os.environ.get("DILATED_REPS", "1"))
    for p in [i % NPROB for i in range(reps * NPROB)]:
        st = probs[p]

        qka_t = qka_pool.tile([128, 2 * SL], FP16)
        nc.sync.dma_start(out=qka_t, in_=qka_ap[p])
        qkb_t = qkb_pool.tile([128, 2 * SL], FP16)
        nc.sync.dma_start(out=qkb_t, in_=qkb_ap[p])
        vp_t = vp_pool.tile([128, NCHUNK * 2 * VW], FP16)
        nc.sync.dma_start(out=vp_t, in_=vp_ap[p])

        pv_ps = None
        for c in range(NCHUNK):
            # S^T chunk c = khi.T @ (qhi+qlo)  +  klo.T @ qhi
            base = (c % 2) * 64
            s_ps = spsum.tile([128, SL], FP32)
            for h in range(2):
                hs = slice(h * 512, (h + 1) * 512)
                nc.tensor.matmul(      # K=128 stacked: khi.T (qhi+qlo)
                    out=s_ps[:, hs],
                    lhsT=qka_t[:, SL + c * 128: SL + (c + 1) * 128],
                    rhs=qka_t[:, hs],
                    start=True, stop=False,
                )
                nc.tensor.matmul(      # K=64 row-packed: klo.T qhi
                    out=s_ps[:, hs],
                    lhsT=qkb_t[base:base + 64, SL + c * 128: SL + (c + 1) * 128],
                    rhs=qkb_t[base:base + 64, hs],
                    start=False, stop=True,
                )

            if c == 1 and prev is not None:
                emit_bcast(prev)
            if c == 2 and prev is not None:
                emit_norm(prev)

            e_t = exp_pool.tile([128, SL], FP16)
            nc.scalar.activation(out=e_t, in_=s_ps, func=EXP)

            if pv_ps is None:
                pv_ps = pvpsum.tile([128, SL], FP32, tag="pv")
            for h in range(2):
                hs = slice(h * 512, (h + 1) * 512)
                nc.tensor.matmul(      # [Vhi | 1].T @ E
                    out=pv_ps[0:VW, hs],
                    lhsT=vp_t[:, c * 2 * VW: c * 2 * VW + VW],
                    rhs=e_t[:, hs],
                    start=(c == 0), stop=False,
                )
                nc.tensor.matmul(      # [Vlo | 0].T @ E
                    out=pv_ps[0:VW, hs],
                    lhsT=vp_t[:, c * 2 * VW + VW: (c + 1) * 2 * VW],
                    rhs=e_t[:, hs],
                    start=False, stop=(c == NCHUNK - 1),
                )

        # evacuate PV psum fast (frees the slot), compute 1/l row
        s_out = sout_pool.tile([VW, SL], FP32)
        nc.vector.tensor_copy(out=s_out, in_=pv_ps[0:VW, :])
        st["s_out"] = s_out
        r_row = rrow_pool.tile([1, SL], FP32)
        nc.vector.reciprocal(out=r_row, in_=s_out[D:D + 1, :])
        st["r_row"] = r_row

        prev = st

    # drain the last problem's epilogue
    emit_bcast(prev)
    emit_norm(prev)


# Cache: the Bass program is identical for every call (and every core).
_CACHED = {}


def _get_program():
    key = os.environ.get("DILATED_REPS", "1")
    if key in _CACHED:
        return _CACHED[key]
    nc = bacc.Bacc("TRN2", target_bir_lowering=False, debug=False)
    qka = nc.dram_tensor("qka", [NPROB, 128, 2 * SL], FP16,
                         kind="ExternalInput").ap()
    qkb = nc.dram_tensor("qkb", [NPROB, 128, 2 * SL], FP16,
                         kind="ExternalInput").ap()
    vp = nc.dram_tensor("vp", [NPROB, 128, NCHUNK * 2 * VW], FP16,
                        kind="ExternalInput").ap()
    out = nc.dram_tensor("out", [NPROB, D, SL], FP32, kind="ExternalOutput").ap()
    from contextlib import ExitStack
    with tile.TileContext(nc) as tc, ExitStack() as ctx:
        _build_tile_program(ctx, tc, out, qka, qkb, vp)
    nc.compile()
    _CACHED[key] = nc
    return nc


# ---------------------------------------------------------------- host glue
def _prep_core(q, k, v, b, j):
    """Build the qka/qkb/vp device inputs for core (b, j). q is pre-scaled."""
    f16 = np.float16
    qka = np.empty((NPROB, 128, 2 * SL), dtype=f16)
    qkb = np.empty((NPROB, 128, 2 * SL), dtype=f16)
    vp = np.empty((NPROB, 128, NCHUNK * 2 * VW), dtype=f16)
    ones = np.ones((SL, 1), np.float32)
    zeros = np.zeros((SL, 1), np.float32)
    for p, (g, head, seg) in enumerate(_problem_list(j)):
        pos = _positions(g, seg)
        qT = q[b, pos, head, :].T  # [64, 1024] fp32, already scaled
        kT = k[b, pos, head, :].T
        qhi = qT.astype(f16)
        qlo = (qT - qhi.astype(np.float32)).astype(f16)
        khi = kT.astype(f16)
        klo = (kT - khi.astype(np.float32)).astype(f16)
        # qka: rows0-63 [qhi | khi], rows64-127 [qlo | khi]
        qka[p, 0:64, 0:SL] = qhi
        qka[p, 64:128, 0:SL] = qlo
        qka[p, 0:64, SL:] = khi
        qka[p, 64:128, SL:] = khi
        # qkb: both halves [qhi | klo] (row-packed correction operands)
        qkb[p, 0:64, 0:SL] = qhi
        qkb[p, 64:128, 0:SL] = qhi
        qkb[p, 0:64, SL:] = klo
        qkb[p, 64:128, SL:] = klo
        vs = v[b, pos, head, :]  # [1024, 64] fp32
        vhi = vs.astype(f16)
        vlo = (vs - vhi.astype(np.float32)).astype(f16)
        vfull = np.concatenate(
            [vhi.astype(np.float32), ones, vlo.astype(np.float32), zeros],
            axis=1)  # [1024, 130]
        vp[p] = (vfull.reshape(NCHUNK, 128, 2 * VW)
                 .transpose(1, 0, 2).reshape(128, NCHUNK * 2 * VW)
                 .astype(f16))
    return {"qka": qka, "qkb": qkb, "vp": vp}


def kernel(query, key, value, _run_kw=None):
    q = np.asarray(query, dtype=np.float32)
    k = np.asarray(key, dtype=np.float32)
    v = np.asarray(value, dtype=np.float32)
    qs = q * SCALE  # fold softmax scale into q

    nc = _get_program()
    in_maps = []
    core_meta = []
    for core in range(N_CORES):
        b, j = divmod(core, NUM_GROUPS)
        in_maps.append(_prep_core(qs, k, v, b, j))
        core_meta.append((b, j))

    kw = dict(_run_kw or {})
    kw.pop("result", None)
    res = bass_utils.run_bass_kernel_spmd(
        nc, in_maps, core_ids=list(range(N_CORES)), **kw)

    out = np.zeros((B, S, H, D), dtype=np.float32)
    for core in range(N_CORES):
        b, j = core_meta[core]
        dev_out = res.results[core]["out"]  # [15, 64, 1024] fp32
        for p, (g, head, seg) in enumerate(_problem_list(j)):
            pos = _positions(g, seg)
            out[b, pos, head, :] = dev_out[p].T
    if _run_kw is not None:
        _run_kw["result"] = res
    return out



# revision 7
# speedup vs baseline: 1.8683x; 1.8683x over previous
"""Dilated attention (LongNet-style) Bass kernel for 8 Trainium2 NeuronCores.

Problem: q,k,v of shape (B=2, S=8192, H=16, D=64) fp32.
4 head-groups x (segment length s, dilation r) with s/r == 1024 for every
group, so the whole computation is 120 identical 1024x1024x64 attention
sub-problems plus a per-(batch, head, channel) sum-normalization.

Sharding: core = b*4 + j owns heads {j, 4+j, 8+j, 12+j} of batch b, i.e.
one head from each group -> 8+4+2+1 = 15 sub-problems per core (perfectly
balanced), and every (batch, head) lives on exactly one core.

The device computes only the O(S^2) part (scores, exp, PV); the O(S)
normalization (1/l, hi+lo fold, channel sums, final scale) runs on the
host in fp64, which is both faster (no cross-partition folds on device —
DVE lanes are partition-locked and sub-32-partition ops at base 64
produce garbage on HW) and more accurate.

Numerics: the final x / sum(x) normalization is badly conditioned; error
sources that are correlated across queries (K and V quantization) are
amplified ~30-100x, so K and V are fed as bf16-hi + lo pairs (~22
effective bits) while Q and E ride single fp32r (~13 effective bits,
uncorrelated errors average out). All matmuls run in fp32r, which
streams at full bf16 rate (1 cycle/row for N>=256).

Per sub-problem on-device (fp32 PSUM accumulation):
  S^T[k,q] = [khi;klo].T @ [q;q]       one stacked K=128 fp32r matmul per
                                       512-col half per key chunk
  E        = exp(S^T), ACT writes fp32r directly
  PV       = [Vhi|1|Vlo].T @ E         M=128-packed: rows 0:64 hi-part,
                                       row 64 = softmax denom l,
                                       rows 65:128 lo-part
  out      = PV copied to SBUF, DMA'd raw; host folds + normalizes.
"""

import os
import numpy as np
import ml_dtypes

import concourse.bass as bass
import concourse.bacc as bacc
import concourse.mybir as mybir
import concourse.tile as tile
from concourse import bass_utils

# ---------------------------------------------------------------- constants
B, S, H, D = 2, 8192, 16, 64
SEGMENT_LENGTHS = [1024, 2048, 4096, 8192]
DILATION_RATES = [1, 2, 4, 8]
NUM_GROUPS = 4
GROUP_HEADS = H // NUM_GROUPS  # 4
SEGS_PER_GROUP = [S // s for s in SEGMENT_LENGTHS]  # [8, 4, 2, 1]
NPROB = sum(SEGS_PER_GROUP)  # 15 problems per core
SL = 1024          # per-problem sequence length (s // r, same for all groups)
NCHUNK = SL // 128  # 8 key chunks
N_CORES = 8
SCALE = 1.0 / np.sqrt(D)

FP32 = mybir.dt.float32
FP32R = mybir.dt.float32r
VW = D + 1  # 65: hi rows plus the l row


def _problem_list(j):
    """15 (group, head, seg) tuples for local head-slot j, head-contiguous."""
    out = []
    for g in range(NUM_GROUPS):
        head = g * GROUP_HEADS + j
        for seg in range(SEGS_PER_GROUP[g]):
            out.append((g, head, seg))
    return out


def _positions(g, seg):
    s, r = SEGMENT_LENGTHS[g], DILATION_RATES[g]
    offset = g % r
    return seg * s + offset + r * np.arange(SL)


# ---------------------------------------------------------------- device IR
def _build_tile_program(ctx, tc, out_ap, qd_ap, kd_ap, vp_ap):
    nc = tc.nc
    EXP = mybir.ActivationFunctionType.Exp

    qk_pool = ctx.enter_context(tc.tile_pool(name="qk", bufs=3))
    k_pool = ctx.enter_context(tc.tile_pool(name="kt", bufs=3))
    vp_pool = ctx.enter_context(tc.tile_pool(name="vp", bufs=3))
    e_pool = ctx.enter_context(tc.tile_pool(name="exps", bufs=3))
    pvsb_pool = ctx.enter_context(tc.tile_pool(name="pvsb", bufs=3))
    spsum = ctx.enter_context(tc.tile_pool(name="spsum", bufs=2, space="PSUM"))
    pvpsum = ctx.enter_context(tc.tile_pool(name="pvpsum", bufs=2, space="PSUM"))

    reps = int(os.environ.get("DILATED_REPS", "1"))
    for p in [i % NPROB for i in range(reps * NPROB)]:
        qk_t = qk_pool.tile([128, SL], FP32R)
        nc.sync.dma_start(out=qk_t[0:D, :], in_=qd_ap[p])
        nc.sync.dma_start(out=qk_t[D:128, :], in_=qd_ap[p])
        k_t = k_pool.tile([128, SL], FP32R)
        nc.sync.dma_start(out=k_t, in_=kd_ap[p])
        vp_t = vp_pool.tile([128, NCHUNK * 128], FP32R)
        nc.sync.dma_start(out=vp_t, in_=vp_ap[p])

        pv_ps = None
        for c in range(NCHUNK):
            # S^T chunk c = khi.T q + klo.T q, one stacked K=128 matmul/half
            s_ps = spsum.tile([128, SL], FP32, tag="s")
            for h in range(2):
                hs = slice(h * 512, (h + 1) * 512)
                nc.tensor.matmul(
                    out=s_ps[:, hs],
                    lhsT=k_t[:, c * 128:(c + 1) * 128],
                    rhs=qk_t[:, hs],
                    start=True, stop=True,
                )

            e_t = e_pool.tile([128, SL], FP32R)
            nc.scalar.activation(out=e_t, in_=s_ps, func=EXP)

            if pv_ps is None:
                pv_ps = pvpsum.tile([128, SL], FP32, tag="pv")
            for h in range(2):
                hs = slice(h * 512, (h + 1) * 512)
                nc.tensor.matmul(      # [Vhi | 1 | Vlo].T @ E
                    out=pv_ps[:, hs],
                    lhsT=vp_t[:, c * 128:(c + 1) * 128],
                    rhs=e_t[:, hs],
                    start=(c == 0), stop=(c == NCHUNK - 1),
                )

        # evacuate PV psum to SBUF and ship it; host does the rest
        pv_sb = pvsb_pool.tile([128, SL], FP32)
        nc.vector.tensor_copy(out=pv_sb, in_=pv_ps)
        nc.gpsimd.dma_start(out=out_ap[p], in_=pv_sb)


# Cache: the Bass program is identical for every call (and every core).
_CACHED = {}


def _get_program():
    key = os.environ.get("DILATED_REPS", "1")
    if key in _CACHED:
        return _CACHED[key]
    nc = bacc.Bacc("TRN2", target_bir_lowering=False, debug=False)
    qd = nc.dram_tensor("qd", [NPROB, D, SL], FP32R,
                        kind="ExternalInput").ap()
    kd = nc.dram_tensor("kd", [NPROB, 128, SL], FP32R,
                        kind="ExternalInput").ap()
    vp = nc.dram_tensor("vp", [NPROB, 128, NCHUNK * 128], FP32R,
                        kind="ExternalInput").ap()
    out = nc.dram_tensor("out", [NPROB, 128, SL], FP32,
                         kind="ExternalOutput").ap()
    from contextlib import ExitStack
    with tile.TileContext(nc) as tc, ExitStack() as ctx:
        _build_tile_program(ctx, tc, out, qd, kd, vp)
    nc.compile()
    _CACHED[key] = nc
    return nc


# ---------------------------------------------------------------- host glue
def _bf16hi(x):
    return x.astype(ml_dtypes.bfloat16).astype(np.float32)


def _prep_core(q, k, v, b, j):
    """Build the qd/kd/vp device inputs for core (b, j). q is pre-scaled."""
    qd = np.empty((NPROB, D, SL), dtype=np.float32)
    kd = np.empty((NPROB, 128, SL), dtype=np.float32)
    vp = np.empty((NPROB, 128, NCHUNK * 128), dtype=np.float32)
    ones = np.ones((SL, 1), np.float32)
    for p, (g, head, seg) in enumerate(_problem_list(j)):
        pos = _positions(g, seg)
        qd[p] = q[b, pos, head, :].T  # [64, 1024] fp32, already scaled
        kT = k[b, pos, head, :].T
        khi = _bf16hi(kT)
        kd[p, 0:D] = khi
        kd[p, D:128] = kT - khi
        vs = v[b, pos, head, :]  # [1024, 64] fp32
        vhi = _bf16hi(vs)
        vlo = vs - vhi
        block = np.concatenate([vhi, ones, vlo[:, 0:D - 1]], axis=1)
        vp[p] = (block.reshape(NCHUNK, 128, 128)
                 .transpose(1, 0, 2).reshape(128, NCHUNK * 128))
    return {"qd": qd, "kd": kd, "vp": vp}


def _postprocess_core(dev_out):
    """[NPROB, 128, SL] raw PV -> [NPROB, D, SL] normalized attention out.

    Folds the packed lo rows into the hi rows, divides by the softmax
    denominator row, then applies the per-(head, channel) 1/(4*sum)
    normalization across each head's segments. fp64 throughout.
    """
    pv = dev_out.astype(np.float64)
    x = pv[:, 0:D, :].copy()
    x[:, 0:D - 1, :] += pv[:, VW:128, :]
    x /= pv[:, D:VW, :]  # divide by l row
    outp = np.empty((NPROB, D, SL), dtype=np.float32)
    i = 0
    for nseg in SEGS_PER_GROUP:
        xs = x[i:i + nseg]  # [nseg, D, SL]
        hsum = xs.sum(axis=(0, 2), keepdims=True)  # [1, D, 1]
        outp[i:i + nseg] = (xs / (NUM_GROUPS * hsum)).astype(np.float32)
        i += nseg
    return outp


def kernel(query, key, value, _run_kw=None):
    q = np.asarray(query, dtype=np.float32)
    k = np.asarray(key, dtype=np.float32)
    v = np.asarray(value, dtype=np.float32)
    qs = q * SCALE  # fold softmax scale into q

    nc = _get_program()
    in_maps = []
    core_meta = []
    for core in range(N_CORES):
        b, j = divmod(core, NUM_GROUPS)
        in_maps.append(_prep_core(qs, k, v, b, j))
        core_meta.append((b, j))

    kw = dict(_run_kw or {})
    kw.pop("result", None)
    res = bass_utils.run_bass_kernel_spmd(
        nc, in_maps, core_ids=list(range(N_CORES)), **kw)

    out = np.zeros((B, S, H, D), dtype=np.float32)
    for core in range(N_CORES):
        b, j = core_meta[core]
        dev_out = _postprocess_core(res.results[core]["out"])
        for p, (g, head, seg) in enumerate(_problem_list(j)):
            pos = _positions(g, seg)
            out[b, pos, head, :] = dev_out[p].T
    if _run_kw is not None:
        _run_kw["result"] = res
    return out
